# revision 7
# baseline (speedup 1.0000x reference)
"""Trainium2 Bass kernel for nn_DifferentiableKalmanFilter (v2).

Strategy
--------
The 4x4 covariance recursion is batch-independent and, by x/y symmetry,
collapses to two scalar gain sequences k_p(t), k_v(t) computed on the host.
Per batch row the filter is a 2-state linear recurrence per coordinate:

    s_t = s_{t-1} @ M_t + z_t * g_t,   s = [p, v],
    M_t = [[1, 0], [dt - k_p(t), 1 - k_v(t)]],  g_t = [k_p(t), k_v(t)]

Unrolled over a chunk of L=126 timesteps this is a matmul with
host-precomputed weights. The tolerance (2e-2) allows a single fp16
product (no hi/lo splitting, no scaling): z, W and the carried state are
plain fp16; PSUM accumulates in f32.

Layout: weights are the stationary operand, the z stack [K, batch] is the
moving operand, so each matmul streams 512 batch columns and yields a
time-major [M, batch] output plane. Output columns 0,1 hold the chunk-end
state [p_end, v_end]; real outputs sit in columns 2..M-1. The chunk
boundary costs no DMA and no extra copy: each chunk's psum accumulates
two matmuls — Wz over its z-only stack (start) and a K=2 carry matmul
(stop) that reads rows 0,1 of the PREVIOUS chunk's fp16 pos out tile
directly. Only [pos copy -> K=2 matmul] sits on the serial chain, so PE
runs one gap-free block and the kernel is jointly PE/DMA-bound.

Outputs: pos stays fp16; vel is quantized to int8 on the way out of PSUM
with a host-baked scale (|v_t| <= max|z| provably — the vel filter is a
convex combination of past z's — so the scale is sound and saturation is
impossible). That cuts output traffic 25%. The host dequantizes.

Per (chunk, coord): 4 z-matmuls + 4 carry matmuls [*, 512], 1 fp16 pos
copy (ACT) + 1 int8 vel copy (DVE) of [*, 1024], 2 output DMAs. Inputs
stream via gpsimd SWDGE three chunks ahead (just-in-time: the DMA
device is FIFO by arrival, so earlier input issue pushes the output
transfers — and the drain — later); outputs issue one chunk late so
their SEQ never stalls holding the HWDGE queue. Startup: chunk-0's x
stack and W0 lead the two HWDGE queues, and a stream of dummy matmuls
(no DMA deps) pre-ramps the PE p-state so the real block runs at full
clock from its first instruction.

Per-core cost model estimate: ~38.5us vs 106.2us baseline (2.76x), with
DMA_ENGINES 29.5us busy and PE 29.0us busy (all matmuls at 2.4GHz),
fully overlapped.

Sharding: pure data parallel over batch across 8 cores (1024 rows/core).
"""

import numpy as np

import concourse.bass as bass
import concourse.tile as tile
from concourse import bacc, mybir
from concourse.bass_utils import run_bass_kernel_spmd

# Problem shape (hardcoded per harness contract)
B = 8192
T = 1024
NCORES = 8
BC = B // NCORES  # 1024 batch rows per core
L = 126  # z rows per full chunk; K = L + 2 <= 128
CH = []
_t0 = 0
while _t0 < T:
    CH.append((_t0, min(L, T - _t0)))
    _t0 += CH[-1][1]
NCH = len(CH)  # 8 full chunks + 16-step tail


# ---------------------------------------------------------------- host math
def _gains(dt, q_pos, q_vel, r_vel):
    """Scalar Kalman gain sequences in float64 (exact vs fp32 reference)."""
    dt = float(dt)
    r_reg = float(np.float32(r_vel) + np.float32(1e-6))
    q_pos = float(q_pos)
    q_vel = float(q_vel)
    a, b, c = 1.0, 0.0, 1.0  # P blocks [[a, b], [b, c]] per coordinate
    kp = np.zeros(T)
    kv = np.zeros(T)
    for t in range(T):
        ap = a + 2 * dt * b + dt * dt * c + q_pos
        bp = b + dt * c
        cp = c + q_vel
        den = cp + r_reg
        kp[t] = bp / den
        kv[t] = cp / den
        a = ap - kp[t] * bp
        b = bp * r_reg / den
        c = cp * r_reg / den
    return kp, kv


def _build_weights(kp, kv, dt):
    """Per-chunk weights W[K, 2, K] (fp16), K = Lc + 2.

    Stack rows: [p_c, v_c, z_0..z_{Lc-1}]. For each plane pl (0=pos,
    1=vel): col 0 = p_end coeffs, col 1 = v_end coeffs, col 2+u = state
    component pl at local step u.
    Full-size chunks are deduped (gains converge -> steady chunks share W).
    """
    dt = float(dt)
    w64 = []
    for t0, Lc in CH:
        K = Lc + 2
        U = np.zeros((K, 2))
        U[0] = (1.0, 0.0)
        U[1] = (0.0, 1.0)
        W = np.zeros((K, 2, K))
        for u in range(Lc):
            t = t0 + u
            M = np.array([[1.0, 0.0], [dt - kp[t], 1.0 - kv[t]]])
            U[: 2 + u] = U[: 2 + u] @ M
            U[2 + u] = (kp[t], kv[t])
            W[: 2 + u + 1, :, 2 + u] = U[: 2 + u + 1]
        W[:, 0, 0:2] = U  # [p_end, v_end] coeff cols (both planes)
        W[:, 1, 0:2] = U
        w64.append(W.astype(np.float16))

    chunk_map = []
    uniq = []
    nfull = sum(1 for _, Lc in CH if Lc == L)
    for c in range(nfull):
        found = None
        for ui, u in enumerate(uniq):
            if np.array_equal(w64[c], w64[u]):
                found = ui
                break
        if found is None:
            uniq.append(c)
            found = len(uniq) - 1
        chunk_map.append(found)

    w_full = np.stack([w64[u] for u in uniq])  # (NU, 128, 2, 128)
    w_last = w64[-1] if CH[-1][1] != L else None  # (Kl, 2, Kl)
    return w_full, w_last, chunk_map


def _split_weights(w_full, w_last, chunk_map):
    """Split into chunk-0 full W, z-part/carry-part for chunks >= 1.

    Chunk 0's carry rows are host-baked into its stack (single matmul);
    later chunks run two accumulating matmuls: Wz over the z-only stack
    and Wc (K=2) over the previous out tile's [p_end, v_end] rows.
    """
    w0 = w_full[chunk_map[0]]  # (128, 2, 128)
    uniq1 = sorted({chunk_map[c] for c in range(1, len(chunk_map))})
    remap = {u: i for i, u in enumerate(uniq1)}
    chunk_map1 = [remap[chunk_map[c]] for c in range(1, len(chunk_map))]
    wz = np.ascontiguousarray(
        np.stack([w_full[u] for u in uniq1]).transpose(1, 0, 2, 3)[2:]
    )  # (126, NU1, 2, 128)
    wc = np.ascontiguousarray(
        np.stack([w_full[u] for u in uniq1]).transpose(1, 0, 2, 3)[0:2]
    )  # (2, NU1, 2, 128)
    if w_last is not None:
        wzl = np.ascontiguousarray(w_last[2:])  # (16, 2, 18)
        wcl = np.ascontiguousarray(w_last[0:2])  # (2, 2, 18)
    else:
        wzl = wcl = None
    return w0, wz, wc, wzl, wcl, chunk_map1


# ---------------------------------------------------------------- bass build
def _build_nc(nu1, chunk_map1, has_last, vsc):
    f32 = mybir.dt.float32
    f16 = mybir.dt.float16

    nc = bacc.Bacc(
        "TRN2",
        target_bir_lowering=False,
        debug=False,
        enable_asserts=False,
    )
    KF = L + 2  # 128
    zin_d = nc.dram_tensor("zin", [2, NCH, KF, BC], f16, kind="ExternalInput").ap()
    w0_d = nc.dram_tensor("w0", [KF, 2, KF], f16, kind="ExternalInput").ap()
    wz_d = nc.dram_tensor("wz", [L, nu1, 2, KF], f16, kind="ExternalInput").ap()
    wc_d = nc.dram_tensor("wc", [2, nu1, 2, KF], f16, kind="ExternalInput").ap()
    if has_last:
        _, Ll = CH[-1]
        wzl_d = nc.dram_tensor("wzl", [Ll, 2, Ll + 2], f16, kind="ExternalInput").ap()
        wcl_d = nc.dram_tensor("wcl", [2, 2, Ll + 2], f16, kind="ExternalInput").ap()
    outp_d = nc.dram_tensor("outp", [2, NCH, L, BC], f16, kind="ExternalOutput").ap()
    outv_d = nc.dram_tensor(
        "outv", [2, NCH, L, BC], mybir.dt.int8, kind="ExternalOutput"
    ).ap()

    with tile.TileContext(nc) as tc:
        with (
            tc.tile_pool(name="wpool", bufs=1) as wpool,
            tc.tile_pool(name="stacks", bufs=1) as spool,
            tc.tile_pool(name="outp", bufs=1) as opool,
            tc.tile_pool(name="mpsum", bufs=4, space="PSUM") as mpsum_pool,
        ):
            # stacks: chunk 0 holds [p_c, v_c | z] (carry host-baked into
            # zin); chunks >= 1 hold z only — their carry contribution is
            # a K=2 matmul reading the previous out tile's rows 0,1.
            # All loads are issued up front: they never wait, and a deep
            # standing queue of input transfers keeps DMA_ENGINES packed.
            stacks = {}

            def make_stack(c, cd):
                Lc = CH[c][1]
                K = Lc + 2 if c == 0 else Lc
                stk = spool.tile(
                    [K, BC], f16, tag=f"stk_{c}_{cd}", name=f"stk_{c}_{cd}"
                )
                if c == 0:
                    eng = nc.scalar if cd else nc.sync
                    eng.dma_start(stk[0:K, :], zin_d[cd, c, 0:K, :])
                else:
                    nc.gpsimd.dma_start(stk[0:K, :], zin_d[cd, c, 2 : 2 + K, :])
                stacks[(c, cd)] = stk

            # warm the ACT activation table off the critical path; the
            # dummy-matmul source region [16:80] is disjoint from the ACT
            # target [0:8] so the PE stream gates only on the memset
            warm = wpool.tile([1, 80], f16)
            nc.vector.memset(warm[:], 0.0)
            nc.scalar.mul(warm[0:1, 0:8], warm[0:1, 0:8], 1.0)
            # pre-ramp the PE p-state: dummy matmuls (no DMA deps) into ONE
            # reused psum tile keep the PE continuously busy through
            # startup, so the first real matmul runs at full clock (the
            # ramp needs 3us of uninterrupted busy)
            psd = mpsum_pool.tile([KF, BC], f32, tag="ps")
            for _ in range(75):
                nc.tensor.matmul(
                    psd[0:1, 0:64], warm[0:1, 16:17], warm[0:1, 16:80],
                    start=True, stop=True,
                )

            # startup: c0_x leads the scalar queue (first HWDGE grant)
            # while w0 leads sync's — the first matmul's inputs transfer
            # back-to-back, nothing queue-jumps them
            w0_t = wpool.tile([KF, 2, KF], f16)
            nc.gpsimd.dma_start(warm[0:1, 8:16], zin_d[0, 0, 0:1, 0:8])
            stk00 = spool.tile([KF, BC], f16, tag="stk_0_0", name="stk_0_0")
            nc.scalar.dma_start(stk00[:], zin_d[0, 0, :, :])
            stacks[(0, 0)] = stk00
            nc.sync.dma_start(w0_t[:], w0_d)
            stk01 = spool.tile([KF, BC], f16, tag="stk_0_1", name="stk_0_1")
            nc.sync.dma_start(stk01[:], zin_d[1, 0, :, :])
            stacks[(0, 1)] = stk01

            wz_t = wpool.tile([L, nu1, 2, KF], f16)
            nc.scalar.dma_start(wz_t[:], wz_d)
            wc_t = wpool.tile([2, nu1, 2, KF], f16)
            nc.sync.dma_start(wc_t[:], wc_d)
            if has_last:
                _, Ll = CH[-1]
                wzl_t = wpool.tile([Ll, 2, Ll + 2], f16)
                nc.scalar.dma_start(wzl_t[:], wzl_d)
                wcl_t = wpool.tile([2, 2, Ll + 2], f16)
                nc.sync.dma_start(wcl_t[:], wcl_d)

            for c in range(1, min(3, NCH)):
                for cd in range(2):
                    make_stack(c, cd)

            outps = {}
            outvs = {}
            for c in range(NCH):
                K = CH[c][1] + 2
                for cd in range(2):
                    outps[(c, cd)] = opool.tile(
                        [K, BC], f16, tag=f"outp_{c}_{cd}", name=f"outp_{c}_{cd}"
                    )
                    outvs[(c, cd)] = opool.tile(
                        [K, BC], mybir.dt.int8,
                        tag=f"outv_{c}_{cd}", name=f"outv_{c}_{cd}",
                    )

            def wzslice(c, pl):
                if CH[c][1] == L:
                    return wz_t[:, chunk_map1[c - 1], pl, :]
                return wzl_t[:, pl, :]

            def wcslice(c, pl):
                if CH[c][1] == L:
                    return wc_t[:, chunk_map1[c - 1], pl, :]
                return wcl_t[:, pl, :]

            for c in range(NCH):
                t0, Lc = CH[c]
                M = Lc + 2
                for cd in range(2):
                    stk = stacks[(c, cd)]
                    pss = []
                    for pl in range(2):
                        ps = mpsum_pool.tile([M, BC], f32, tag="ps")
                        for h in range(2):
                            hsl = slice(h * 512, (h + 1) * 512)
                            if c == 0:
                                nc.tensor.matmul(
                                    ps[:, hsl], w0_t[:, pl, :], stk[:, hsl],
                                    start=True, stop=True,
                                )
                            else:
                                nc.tensor.matmul(
                                    ps[:, hsl], wzslice(c, pl), stk[:, hsl],
                                    start=True, stop=False,
                                )
                                # carry: prev out tile rows 0,1 of pos plane
                                nc.tensor.matmul(
                                    ps[:, hsl], wcslice(c, pl),
                                    outps[(c - 1, cd)][0:2, hsl],
                                    start=False, stop=True,
                                )
                        pss.append(ps)
                    # pos copy on ACT (feeds next chunk's carry matmuls);
                    # vel quantizes to int8 on DVE (|v| <= max|z| makes the
                    # host-computed scale sound, so no saturation)
                    nc.scalar.mul(outps[(c, cd)][:], pss[0][:], 1.0)
                    nc.vector.tensor_scalar_mul(outvs[(c, cd)][:], pss[1][:], vsc)
                # prefetch inputs three chunks ahead
                if c + 3 < NCH:
                    for cd in range(2):
                        make_stack(c + 3, cd)
                # out DMAs are emitted one chunk late: their copies are
                # long done by then, so the issuing SEQ never stalls
                # holding the queue while a copy drains
                for cp in ([c - 1] if c else []) + ([c] if c == NCH - 1 else []):
                    for cd in range(2):
                        Lc = CH[cp][1]
                        eng = nc.scalar if cd else nc.sync
                        oth = nc.sync if cd else nc.scalar
                        eng.dma_start(
                            outp_d[cd, cp, 0:Lc, :], outps[(cp, cd)][2 : 2 + Lc, :]
                        )
                        oth.dma_start(
                            outv_d[cd, cp, 0:Lc, :], outvs[(cp, cd)][2 : 2 + Lc, :]
                        )
    nc.compile()
    return nc


# ---------------------------------------------------------------- entry
def _prepare(pred_vel, dt, p0, q_pos, q_vel, r_vel):
    kp, kv = _gains(dt, q_pos, q_vel, r_vel)
    w_full, w_last, chunk_map = _build_weights(kp, kv, dt)
    w0, wz, wc, wzl, wcl, chunk_map1 = _split_weights(w_full, w_last, chunk_map)
    nu1 = wz.shape[1]

    pred_vel = np.asarray(pred_vel, dtype=np.float32)
    p0 = np.asarray(p0, dtype=np.float32)
    vsc = 126.0 / max(float(np.abs(pred_vel).max()), 1e-30)
    in_maps = []
    for i in range(NCORES):
        pv = pred_vel[i * BC : (i + 1) * BC]  # (BC, T, 2)
        zt = np.ascontiguousarray(pv.transpose(2, 1, 0)).astype(np.float16)
        zin = np.zeros((2, NCH, L + 2, BC), dtype=np.float16)
        for c, (t0, Lc) in enumerate(CH):
            zin[:, c, 2 : 2 + Lc, :] = zt[:, t0 : t0 + Lc, :]
        # chunk 0 carry rows: p = p0, v = 0
        zin[:, 0, 0, :] = p0[i * BC : (i + 1) * BC].T.astype(np.float16)
        m = {"zin": zin, "w0": w0, "wz": wz, "wc": wc}
        if wzl is not None:
            m["wzl"] = wzl
            m["wcl"] = wcl
        in_maps.append(m)
    return nu1, chunk_map1, wzl is not None, vsc, in_maps


def run(pred_vel, dt, p0, q_pos, q_vel, r_vel, trace=False, **spmd_kwargs):
    nu1, chunk_map1, has_last, vsc, in_maps = _prepare(
        pred_vel, dt, p0, q_pos, q_vel, r_vel
    )
    nc = _build_nc(nu1, chunk_map1, has_last, vsc)
    res = run_bass_kernel_spmd(
        nc, in_maps, core_ids=list(range(NCORES)), trace=trace, **spmd_kwargs
    )
    pos = np.empty((B, T, 2), dtype=np.float32)
    vel = np.empty((B, T, 2), dtype=np.float32)
    inv = 1.0 / vsc
    for i in range(NCORES):
        op = res.results[i]["outp"]  # (2, NCH, 126, BC) fp16
        ov = res.results[i]["outv"]  # (2, NCH, 126, BC) int8
        sl = slice(i * BC, (i + 1) * BC)
        for c, (t0, Lc) in enumerate(CH):
            for cd in range(2):
                pos[sl, t0 : t0 + Lc, cd] = (
                    op[cd, c, 0:Lc, :].astype(np.float32).T
                )
                vel[sl, t0 : t0 + Lc, cd] = (
                    ov[cd, c, 0:Lc, :].astype(np.float32).T * inv
                )
    return (pos, vel), res


def kernel(pred_vel, dt, p0, q_pos, q_vel, r_vel):
    (pos, vel), _ = run(pred_vel, dt, p0, q_pos, q_vel, r_vel, trace=False)
    return pos, vel
